# revision 11
# baseline (speedup 1.0000x reference)
"""Trainium2 Bass kernel for nn_KernelConv2D (per-pixel dynamic depthwise 5x5
conv + leaky ReLU), SPMD across 8 NeuronCores.

Problem (hardcoded shapes):
  feat_in: (2, 32, 256, 256) f32
  kernel:  (2, 800, 256, 256) f32   # 800 = 32 channels * 25 taps, c-major
  out[n,c,h,w] = lrelu_0.2( sum_{i,j} pad_edge(feat)[n,c,h+i,w+j]
                            * kernel[n, c*25+i*5+j, h, w] )

Sharding: 8 shards = (n in {0,1}) x (h-quarter of 64 rows). Replicated padding
is materialized host-side while building each shard's feat slice (halo rows
come from the full tensor; true edges are edge-replicated), so no collectives
are needed on-device.

On-core layout: SBUF partitions = (c, hb) with hb = one of 4 16-row h-blocks;
free dim = (h, w) spatial. Tap shifts are free-dim AP offsets into a
halo-padded feat tile. Per tap: one tensor-tensor multiply and one accumulate,
split across the Vector (DVE) and GpSimd engines so both run concurrently;
the kernel-tensor taps (2 MB each) stream from HBM double-buffered.
"""

import sys

for _p in ("/opt/trn_rl_repo",):
    if _p not in sys.path:
        sys.path.insert(0, _p)

import numpy as np

KSIZE = 5
PAD = 2
N, C, H, W = 2, 32, 256, 256
KK = KSIZE * KSIZE
NCORES = 8

HC = H // 4  # 64 output rows per core
HB = 4  # h sub-blocks per core (partitions = C * HB = 128)
HH = HC // HB  # 16 rows per block
WP = W + 2 * PAD  # 260
HP = HH + KSIZE - 1  # 20 rows per block incl halo
FREE_K = HH * W  # 4096
FREE_F = HP * WP  # 5200

# Taps computed on GpSimd (the rest on DVE). GpSimd 2-input elementwise is
# ~2x slower than DVE, so it gets ~1/3 of the taps.
GPS_TAPS = frozenset(t for t in range(KK) if t % 3 == 1) | {24}

LAST_EXEC_NS = None
LAST_RESULTS = None

_NC = None


KBUFS = 3  # kernel-tap ring depth per compute engine


def _build_nc():
    """Raw-bass (explicit blocks + semaphores) SPMD kernel for one core.

    Engine roles:
      ACT  (scalar): feat DMA in, final output DMA out (HWDGE ring 2)
      SP   (sync):   streams the 25 kernel-tap DMAs (HWDGE ring 1),
                     ring-buffered KBUFS deep per compute engine
      DVE  (vector): taps: in-place multiply + accumulate; final merge+lrelu
      POOL (gpsimd): its share of taps, same pattern

    Raw bass is used instead of Tile because walrus only allows one sync
    wait per instruction; explicit wait_ge()s are standalone sequencer
    instructions, which sidesteps that limit.
    """
    import concourse.bass as bass
    import concourse.mybir as mybir
    from contextlib import ExitStack

    nc = bass.Bass()
    dt = mybir.dt.float32
    feat_d = nc.declare_dram_parameter("feat", [C, HB, HP, WP], dt, isOutput=False)
    ker_d = nc.declare_dram_parameter("ker", [C * KK, HC, W], dt, isOutput=False)
    out_d = nc.declare_dram_parameter("out", [C, HC, W], dt, isOutput=True)

    # DRAM views. Partition order is (c, hb); DMA matches element order.
    kview = ker_d[:, :, :].rearrange(
        "(c t) (hb q) w -> c hb t (q w)", t=KK, hb=HB
    )  # (C, HB, KK, FREE_K); per-(c,hb) a 16 KiB contiguous run per tap
    fview = feat_d[:, :, :, :].rearrange("c hb hp wp -> (c hb) (hp wp)")
    oview = out_d[:, :, :].rearrange("c (hb q) w -> (c hb) (q w)", hb=HB)

    dve_taps = [t for t in range(KK) if t not in GPS_TAPS]
    gps_taps = [t for t in range(KK) if t in GPS_TAPS]
    n_d, n_g = len(dve_taps), len(gps_taps)

    with ExitStack() as ctx:
        feat_s = ctx.enter_context(nc.sbuf_tensor([128, FREE_F], dt))
        kbuf_d = ctx.enter_context(nc.sbuf_tensor([128, KBUFS * FREE_K], dt))
        kbuf_g = ctx.enter_context(nc.sbuf_tensor([128, KBUFS * FREE_K], dt))
        acc_d = ctx.enter_context(nc.sbuf_tensor([128, FREE_K], dt))
        acc_g = ctx.enter_context(nc.sbuf_tensor([128, FREE_K], dt))
        out_s = ctx.enter_context(nc.sbuf_tensor([128, FREE_K], dt))

        fd_sem = ctx.enter_context(nc.semaphore("fd"))
        # One DMA-completion sem per ring slot: completions of different
        # dma_starts on one sem can interleave, so a shared counting sem
        # cannot order slots. Per-slot sems are unambiguous because the
        # consumption sem serializes refills of any single slot.
        kd_sems = [
            ctx.enter_context(nc.semaphore(f"kd{s}")) for s in range(KBUFS)
        ]
        kg_sems = [
            ctx.enter_context(nc.semaphore(f"kg{s}")) for s in range(KBUFS)
        ]
        # Per-engine op counters: each compute op incs its engine's sem by 1.
        # Used for (a) same-engine RAW ordering (deep pipelines), (b) SP's
        # ring-slot refill gating, (c) cross-engine ordering at the tail.
        dve_sem = ctx.enter_context(nc.semaphore("dve_ops"))
        pool_sem = ctx.enter_context(nc.semaphore("pool_ops"))
        odma = ctx.enter_context(nc.semaphore("odma"))
        block = ctx.enter_context(nc.Block())

        feat_v = feat_s[:, :].rearrange("p (hp wp) -> p hp wp", wp=WP)

        def kslot(buf, k):
            s = (k % KBUFS) * FREE_K
            return buf[:, s : s + FREE_K]

        # op count on an engine after tap k completes: mult+add per tap,
        # minus the first tap's fused mult-into-acc (single op)
        def ops_after_tap(k):
            return 2 * k + 1

        n_dve_ops = ops_after_tap(n_d - 1) + 2  # + merge + lrelu
        n_gps_ops = ops_after_tap(n_g - 1)

        @block.scalar
        def _(scalar):
            scalar.dma_start(feat_s[:, :], fview).then_inc(fd_sem, 16)
            scalar.wait_ge(dve_sem, n_dve_ops)
            scalar.dma_start(oview, out_s[:, :]).then_inc(odma, 16)
            scalar.wait_ge(odma, 16)

        @block.sync
        def _(sync):
            d_i = g_i = 0
            for t in range(KK):
                if t in GPS_TAPS:
                    if g_i >= KBUFS:
                        sync.wait_ge(pool_sem, ops_after_tap(g_i - KBUFS))
                    sync.dma_start(kslot(kbuf_g, g_i), kview[:, :, t, :]).then_inc(
                        kg_sems[g_i % KBUFS], 16
                    )
                    g_i += 1
                else:
                    if d_i >= KBUFS:
                        sync.wait_ge(dve_sem, ops_after_tap(d_i - KBUFS))
                    sync.dma_start(kslot(kbuf_d, d_i), kview[:, :, t, :]).then_inc(
                        kd_sems[d_i % KBUFS], 16
                    )
                    d_i += 1

        def tap_ops(eng, taps, kbuf, ksems, own_sem, acc):
            eng.wait_ge(fd_sem, 16)
            nops = 0
            for k, t in enumerate(taps):
                i, j = divmod(t, KSIZE)
                fap = feat_v[:, i : i + HH, j : j + W]
                kt = kslot(kbuf, k)
                kap = kt.rearrange("p (h w) -> p h w", w=W)
                eng.wait_ge(ksems[k % KBUFS], 16 * (k // KBUFS + 1))
                if k == 0:
                    acc3 = acc[:, :].rearrange("p (h w) -> p h w", w=W)
                    eng.tensor_mul(out=acc3, in0=fap, in1=kap).then_inc(own_sem, 1)
                    nops += 1
                else:
                    eng.tensor_mul(out=kap, in0=fap, in1=kap).then_inc(own_sem, 1)
                    nops += 1
                    # same-engine RAW: wait for the multiply before the add
                    eng.wait_ge(own_sem, nops)
                    eng.tensor_add(out=acc[:, :], in0=acc[:, :], in1=kt).then_inc(
                        own_sem, 1
                    )
                    nops += 1
            return nops

        @block.gpsimd
        def _(gpsimd):
            tap_ops(gpsimd, gps_taps, kbuf_g, kg_sems, pool_sem, acc_g)

        @block.vector
        def _(vector):
            nops = tap_ops(vector, dve_taps, kbuf_d, kd_sems, dve_sem, acc_d)
            vector.wait_ge(pool_sem, n_gps_ops)
            vector.wait_ge(dve_sem, nops)
            vector.tensor_add(
                out=acc_d[:, :], in0=acc_d[:, :], in1=acc_g[:, :]
            ).then_inc(dve_sem, 1)
            nops += 1
            vector.wait_ge(dve_sem, nops)
            # leaky relu: max(0.2*x, x) in one fused op
            vector.scalar_tensor_tensor(
                out=out_s[:, :],
                in0=acc_d[:, :],
                scalar=0.2,
                in1=acc_d[:, :],
                op0=mybir.AluOpType.mult,
                op1=mybir.AluOpType.max,
            ).then_inc(dve_sem, 1)
            nops += 1
            assert nops == n_dve_ops, (nops, n_dve_ops)

    nc.finalize()
    return nc


def _get_nc():
    global _NC
    if _NC is None:
        _NC = _build_nc()
    return _NC


def _shard_inputs(feat_in, kernel):
    in_maps = []
    for core in range(NCORES):
        n, hq = divmod(core, 4)
        h0 = hq * HC
        lo = max(0, h0 - PAD)
        hi = min(H, h0 + HC + PAD)
        top = PAD - (h0 - lo)
        bot = PAD - (hi - (h0 + HC))
        fpad = np.pad(
            feat_in[n, :, lo:hi, :], ((0, 0), (top, bot), (PAD, PAD)), mode="edge"
        )  # (C, HC+4, WP)
        fblocks = np.ascontiguousarray(
            np.stack([fpad[:, hb * HH : hb * HH + HP, :] for hb in range(HB)], axis=1)
        )  # (C, HB, HP, WP)
        kshard = np.ascontiguousarray(kernel[n, :, h0 : h0 + HC, :])  # (C*KK, HC, W)
        in_maps.append({"feat": fblocks, "ker": kshard})
    return in_maps


def kernel(feat_in, kernel, _trace=False, _trace_kwargs=None):
    global LAST_EXEC_NS, LAST_RESULTS
    from concourse.bass_utils import run_bass_kernel_spmd

    feat_in = np.ascontiguousarray(np.asarray(feat_in), dtype=np.float32)
    kernel_np = np.ascontiguousarray(np.asarray(kernel), dtype=np.float32)
    assert feat_in.shape == (N, C, H, W), feat_in.shape
    assert kernel_np.shape == (N, C * KK, H, W), kernel_np.shape

    nc = _get_nc()
    in_maps = _shard_inputs(feat_in, kernel_np)
    kwargs = dict(_trace_kwargs or {})
    res = run_bass_kernel_spmd(
        nc, in_maps, core_ids=list(range(NCORES)), trace=_trace, **kwargs
    )
    LAST_EXEC_NS = res.exec_time_ns
    LAST_RESULTS = res

    out = np.empty((N, C, H, W), dtype=np.float32)
    for core in range(NCORES):
        n, hq = divmod(core, 4)
        h0 = hq * HC
        out[n, :, h0 : h0 + HC, :] = res.results[core]["out"]
    return out


# revision 14
# speedup vs baseline: 1.2523x; 1.2523x over previous
"""Trainium2 Bass kernel for nn_KernelConv2D (per-pixel dynamic depthwise 5x5
conv + leaky ReLU), SPMD across 8 NeuronCores.

Problem (hardcoded shapes):
  feat_in: (2, 32, 256, 256) f32
  kernel:  (2, 800, 256, 256) f32   # 800 = 32 channels * 25 taps, c-major
  out[n,c,h,w] = lrelu_0.2( sum_{i,j} pad_edge(feat)[n,c,h+i,w+j]
                            * kernel[n, c*25+i*5+j, h, w] )

Sharding: 8 shards = (n in {0,1}) x (h-quarter of 64 rows). Replicated padding
is materialized host-side while building each shard's feat slice (halo rows
come from the full tensor; true edges are edge-replicated), so no collectives
are needed on-device.

On-core layout: SBUF partitions = (c, hb) with hb = one of 4 16-row h-blocks;
free dim = (h, w) spatial. Tap shifts are free-dim AP offsets into a
halo-padded feat tile. Per tap: one tensor-tensor multiply and one accumulate,
split across the Vector (DVE) and GpSimd engines so both run concurrently;
the kernel-tensor taps (2 MB each) stream from HBM double-buffered.
"""

import sys

for _p in ("/opt/trn_rl_repo",):
    if _p not in sys.path:
        sys.path.insert(0, _p)

import numpy as np

KSIZE = 5
PAD = 2
N, C, H, W = 2, 32, 256, 256
KK = KSIZE * KSIZE
NCORES = 8

HC = H // 4  # 64 output rows per core
HB = 4  # h sub-blocks per core (partitions = C * HB = 128)
HH = HC // HB  # 16 rows per block
WP = W + 2 * PAD  # 260
HP = HH + KSIZE - 1  # 20 rows per block incl halo
FREE_K = HH * W  # 4096
FREE_F = HP * WP  # 5200

# Taps computed on GpSimd (the rest on DVE). HW-measured: concurrent DVE and
# GpSimd 2-input tensor ops serialize on the shared SBUF port pair (both drop
# to ~12.8us per op vs 4.4/8.8 alone), so GpSimd gets NO taps.
GPS_TAPS = frozenset()

LAST_EXEC_NS = None
LAST_RESULTS = None

_NC = None


KBUFS = 3  # kernel-tap ring depth per compute engine


def _build_nc():
    """Raw-bass (explicit blocks + semaphores) SPMD kernel for one core.

    Engine roles:
      ACT  (scalar): feat DMA in, final output DMA out (HWDGE ring 2)
      SP   (sync):   streams the 25 kernel-tap DMAs (HWDGE ring 1),
                     ring-buffered KBUFS deep per compute engine
      DVE  (vector): taps: in-place multiply + accumulate; final merge+lrelu
      POOL (gpsimd): its share of taps, same pattern

    Raw bass is used instead of Tile because walrus only allows one sync
    wait per instruction; explicit wait_ge()s are standalone sequencer
    instructions, which sidesteps that limit.
    """
    import concourse.bass as bass
    import concourse.mybir as mybir
    from contextlib import ExitStack

    nc = bass.Bass()
    dt = mybir.dt.float32
    feat_d = nc.declare_dram_parameter("feat", [C, HB, HP, WP], dt, isOutput=False)
    ker_d = nc.declare_dram_parameter("ker", [C * KK, HC, W], dt, isOutput=False)
    out_d = nc.declare_dram_parameter("out", [C, HC, W], dt, isOutput=True)

    # DRAM views. Partition order is (c, hb); DMA matches element order.
    kview = ker_d[:, :, :].rearrange(
        "(c t) (hb q) w -> c hb t (q w)", t=KK, hb=HB
    )  # (C, HB, KK, FREE_K); per-(c,hb) a 16 KiB contiguous run per tap
    fview = feat_d[:, :, :, :].rearrange("c hb hp wp -> (c hb) (hp wp)")
    oview = out_d[:, :, :].rearrange("c (hb q) w -> (c hb) (q w)", hb=HB)

    dve_taps = [t for t in range(KK) if t not in GPS_TAPS]
    gps_taps = [t for t in range(KK) if t in GPS_TAPS]
    n_d, n_g = len(dve_taps), len(gps_taps)

    with ExitStack() as ctx:
        feat_s = ctx.enter_context(nc.sbuf_tensor([128, FREE_F], dt))
        kbuf_d = ctx.enter_context(nc.sbuf_tensor([128, KBUFS * FREE_K], dt))
        kbuf_g = ctx.enter_context(nc.sbuf_tensor([128, KBUFS * FREE_K], dt))
        acc_d = ctx.enter_context(nc.sbuf_tensor([128, FREE_K], dt))
        acc_g = ctx.enter_context(nc.sbuf_tensor([128, FREE_K], dt))
        out_s = ctx.enter_context(nc.sbuf_tensor([128, FREE_K], dt))

        fd_sem = ctx.enter_context(nc.semaphore("fd"))
        # One DMA-completion sem per ring slot: completions of different
        # dma_starts on one sem can interleave, so a shared counting sem
        # cannot order slots. Per-slot sems are unambiguous because the
        # consumption sem serializes refills of any single slot.
        kd_sems = [
            ctx.enter_context(nc.semaphore(f"kd{s}")) for s in range(KBUFS)
        ]
        kg_sems = [
            ctx.enter_context(nc.semaphore(f"kg{s}")) for s in range(KBUFS)
        ]
        # Per-engine op counters: each compute op incs its engine's sem by 1.
        # Used for (a) same-engine RAW ordering (deep pipelines), (b) SP's
        # ring-slot refill gating, (c) cross-engine ordering at the tail.
        dve_sem = ctx.enter_context(nc.semaphore("dve_ops"))
        pool_sem = ctx.enter_context(nc.semaphore("pool_ops"))
        odma = ctx.enter_context(nc.semaphore("odma"))
        block = ctx.enter_context(nc.Block())

        feat_v = feat_s[:, :].rearrange("p (hp wp) -> p hp wp", wp=WP)

        def kslot(buf, k):
            s = (k % KBUFS) * FREE_K
            return buf[:, s : s + FREE_K]

        # op count on an engine after tap k completes: mult+add per tap,
        # minus the first tap's fused mult-into-acc (single op)
        def ops_after_tap(k):
            return 2 * k + 1

        # + merge (only if GpSimd has taps) + lrelu
        n_dve_ops = ops_after_tap(n_d - 1) + (2 if n_g else 1)
        n_gps_ops = ops_after_tap(n_g - 1) if n_g else 0

        @block.scalar
        def _(scalar):
            scalar.dma_start(feat_s[:, :], fview).then_inc(fd_sem, 16)
            scalar.wait_ge(dve_sem, n_dve_ops)
            scalar.dma_start(oview, out_s[:, :]).then_inc(odma, 16)
            scalar.wait_ge(odma, 16)

        @block.sync
        def _(sync):
            d_i = g_i = 0
            for t in range(KK):
                if t in GPS_TAPS:
                    if g_i >= KBUFS:
                        sync.wait_ge(pool_sem, ops_after_tap(g_i - KBUFS))
                    sync.dma_start(kslot(kbuf_g, g_i), kview[:, :, t, :]).then_inc(
                        kg_sems[g_i % KBUFS], 16
                    )
                    g_i += 1
                else:
                    if d_i >= KBUFS:
                        sync.wait_ge(dve_sem, ops_after_tap(d_i - KBUFS))
                    sync.dma_start(kslot(kbuf_d, d_i), kview[:, :, t, :]).then_inc(
                        kd_sems[d_i % KBUFS], 16
                    )
                    d_i += 1

        def tap_ops(eng, taps, kbuf, ksems, own_sem, acc):
            eng.wait_ge(fd_sem, 16)
            nops = 0
            for k, t in enumerate(taps):
                i, j = divmod(t, KSIZE)
                fap = feat_v[:, i : i + HH, j : j + W]
                kt = kslot(kbuf, k)
                kap = kt.rearrange("p (h w) -> p h w", w=W)
                eng.wait_ge(ksems[k % KBUFS], 16 * (k // KBUFS + 1))
                if k == 0:
                    acc3 = acc[:, :].rearrange("p (h w) -> p h w", w=W)
                    eng.tensor_mul(out=acc3, in0=fap, in1=kap).then_inc(own_sem, 1)
                    nops += 1
                else:
                    eng.tensor_mul(out=kap, in0=fap, in1=kap).then_inc(own_sem, 1)
                    nops += 1
                    # same-engine RAW: wait for the multiply before the add
                    eng.wait_ge(own_sem, nops)
                    eng.tensor_add(out=acc[:, :], in0=acc[:, :], in1=kt).then_inc(
                        own_sem, 1
                    )
                    nops += 1
            return nops

        if n_g:

            @block.gpsimd
            def _(gpsimd):
                tap_ops(gpsimd, gps_taps, kbuf_g, kg_sems, pool_sem, acc_g)

        @block.vector
        def _(vector):
            nops = tap_ops(vector, dve_taps, kbuf_d, kd_sems, dve_sem, acc_d)
            if n_g:
                vector.wait_ge(pool_sem, n_gps_ops)
                vector.wait_ge(dve_sem, nops)
                vector.tensor_add(
                    out=acc_d[:, :], in0=acc_d[:, :], in1=acc_g[:, :]
                ).then_inc(dve_sem, 1)
                nops += 1
            vector.wait_ge(dve_sem, nops)
            # leaky relu: max(0.2*x, x) in one fused op
            vector.scalar_tensor_tensor(
                out=out_s[:, :],
                in0=acc_d[:, :],
                scalar=0.2,
                in1=acc_d[:, :],
                op0=mybir.AluOpType.mult,
                op1=mybir.AluOpType.max,
            ).then_inc(dve_sem, 1)
            nops += 1
            assert nops == n_dve_ops, (nops, n_dve_ops)

    nc.finalize()
    return nc


def _get_nc():
    global _NC
    if _NC is None:
        _NC = _build_nc()
    return _NC


def _shard_inputs(feat_in, kernel):
    in_maps = []
    for core in range(NCORES):
        n, hq = divmod(core, 4)
        h0 = hq * HC
        lo = max(0, h0 - PAD)
        hi = min(H, h0 + HC + PAD)
        top = PAD - (h0 - lo)
        bot = PAD - (hi - (h0 + HC))
        fpad = np.pad(
            feat_in[n, :, lo:hi, :], ((0, 0), (top, bot), (PAD, PAD)), mode="edge"
        )  # (C, HC+4, WP)
        fblocks = np.ascontiguousarray(
            np.stack([fpad[:, hb * HH : hb * HH + HP, :] for hb in range(HB)], axis=1)
        )  # (C, HB, HP, WP)
        kshard = np.ascontiguousarray(kernel[n, :, h0 : h0 + HC, :])  # (C*KK, HC, W)
        in_maps.append({"feat": fblocks, "ker": kshard})
    return in_maps


def kernel(feat_in, kernel, _trace=False, _trace_kwargs=None):
    global LAST_EXEC_NS, LAST_RESULTS
    from concourse.bass_utils import run_bass_kernel_spmd

    feat_in = np.ascontiguousarray(np.asarray(feat_in), dtype=np.float32)
    kernel_np = np.ascontiguousarray(np.asarray(kernel), dtype=np.float32)
    assert feat_in.shape == (N, C, H, W), feat_in.shape
    assert kernel_np.shape == (N, C * KK, H, W), kernel_np.shape

    nc = _get_nc()
    in_maps = _shard_inputs(feat_in, kernel_np)
    kwargs = dict(_trace_kwargs or {})
    res = run_bass_kernel_spmd(
        nc, in_maps, core_ids=list(range(NCORES)), trace=_trace, **kwargs
    )
    LAST_EXEC_NS = res.exec_time_ns
    LAST_RESULTS = res

    out = np.empty((N, C, H, W), dtype=np.float32)
    for core in range(NCORES):
        n, hq = divmod(core, 4)
        h0 = hq * HC
        out[n, :, h0 : h0 + HC, :] = res.results[core]["out"]
    return out


# revision 17
# speedup vs baseline: 1.6054x; 1.2820x over previous
"""Trainium2 Bass kernel for nn_KernelConv2D (per-pixel dynamic depthwise 5x5
conv + leaky ReLU), SPMD across 8 NeuronCores.

Problem (hardcoded shapes):
  feat_in: (2, 32, 256, 256) f32
  kernel:  (2, 800, 256, 256) f32   # 800 = 32 channels * 25 taps, c-major
  out[n,c,h,w] = lrelu_0.2( sum_{i,j} pad_edge(feat)[n,c,h+i,w+j]
                            * kernel[n, c*25+i*5+j, h, w] )

Sharding: 8 shards = (n in {0,1}) x (h-quarter of 64 rows). Replicated padding
is materialized host-side while building each shard's feat slice (halo rows
come from the full tensor; true edges are edge-replicated), so no collectives
are needed on-device.

On-core layout: SBUF partitions = (c, hb) with hb = one of 4 16-row h-blocks;
free dim = (h, w) spatial. Tap shifts are free-dim AP offsets into a
halo-padded feat tile. Per tap: one tensor-tensor multiply and one accumulate,
split across the Vector (DVE) and GpSimd engines so both run concurrently;
the kernel-tensor taps (2 MB each) stream from HBM double-buffered.
"""

import sys

for _p in ("/opt/trn_rl_repo",):
    if _p not in sys.path:
        sys.path.insert(0, _p)

import numpy as np

KSIZE = 5
PAD = 2
N, C, H, W = 2, 32, 256, 256
KK = KSIZE * KSIZE
NCORES = 8

HC = H // 4  # 64 output rows per core
HB = 4  # h sub-blocks per core (partitions = C * HB = 128)
HH = HC // HB  # 16 rows per block
WP = W + 2 * PAD  # 260
HP = HH + KSIZE - 1  # 20 rows per block incl halo
FREE_K = HH * W  # 4096
FREE_F = HP * WP  # 5200

# Taps whose accumulation runs on DVE (tensor_add). The rest accumulate on
# TensorE via identity-matmul into PSUM (fp32-exact, 4 cyc/row), which runs
# concurrently with DVE: PE has its own SBUF read ports. GpSimd is NOT used:
# HW-measured, concurrent DVE+GpSimd 2-input tensor ops serialize on the
# shared SBUF port pair (both drop to ~12.8us vs 4.4/8.8 alone).
DVE_ACC_TAPS = frozenset({1, 6, 11, 16, 21})
KBUFS = 5  # kernel-tap ring depth
PBANK = 512  # fp32 PSUM bank, also max fp32 matmul moving dim

LAST_EXEC_NS = None
LAST_RESULTS = None

_NC = None


def _build_nc():
    """Raw-bass (explicit blocks + semaphores) SPMD kernel for one core.

    Engine roles:
      ACT  (scalar): feat + identity DMA in, odd-tap DMAs, output DMA out
      SP   (sync):   even-tap DMAs (the two HWDGE rings split the stream)
      DVE  (vector): all 25 multiplies (in-place into the tap tile), plus
                     accumulation for DVE_ACC_TAPS; final merge + leaky relu
      PE   (tensor): accumulation of the remaining taps: identity-matmul of
                     each product tile into PSUM (fp32, exact)

    Raw bass is used instead of Tile because walrus only allows one sync
    wait per instruction; explicit wait_ge()s are standalone sequencer
    instructions, which sidesteps that limit.
    """
    import concourse.bass as bass
    import concourse.mybir as mybir
    from contextlib import ExitStack

    nc = bass.Bass()
    dt = mybir.dt.float32
    feat_d = nc.declare_dram_parameter("feat", [C, HB, HP, WP], dt, isOutput=False)
    ker_d = nc.declare_dram_parameter("ker", [C * KK, HC, W], dt, isOutput=False)
    id_d = nc.declare_dram_parameter("ident", [128, 128], dt, isOutput=False)
    out_d = nc.declare_dram_parameter("out", [C, HC, W], dt, isOutput=True)

    # DRAM views. Partition order is (c, hb); DMA matches element order.
    kview = ker_d[:, :, :].rearrange(
        "(c t) (hb q) w -> c hb t (q w)", t=KK, hb=HB
    )  # (C, HB, KK, FREE_K); per-(c,hb) a 16 KiB contiguous run per tap
    fview = feat_d[:, :, :, :].rearrange("c hb hp wp -> (c hb) (hp wp)")
    oview = out_d[:, :, :].rearrange("c (hb q) w -> (c hb) (q w)", hb=HB)

    pe_taps = [t for t in range(KK) if t not in DVE_ACC_TAPS]
    n_pe = len(pe_taps)
    first_acc = min(DVE_ACC_TAPS)
    n_banks = FREE_K // PBANK

    # dve_sem bookkeeping (each DVE op incs it by 1)
    mult_done = {}  # value after tap t's multiply
    tap_done = {}  # value after tap t's last DVE op
    pe_count = {}  # pe_sem value after PE finishes tap t
    ops = 0
    npe = 0
    for t in range(KK):
        ops += 1
        mult_done[t] = ops
        if t in DVE_ACC_TAPS and t != first_acc:
            ops += 1
        tap_done[t] = ops
        if t not in DVE_ACC_TAPS:
            npe += 1
            pe_count[t] = npe
    n_dve_ops = ops + 2  # + merge(psum) + lrelu

    with ExitStack() as ctx:
        feat_s = ctx.enter_context(nc.sbuf_tensor([128, FREE_F], dt))
        kbuf = ctx.enter_context(nc.sbuf_tensor([128, KBUFS * FREE_K], dt))
        acc_d = ctx.enter_context(nc.sbuf_tensor([128, FREE_K], dt))
        out_s = ctx.enter_context(nc.sbuf_tensor([128, FREE_K], dt))
        ident_s = ctx.enter_context(nc.sbuf_tensor([128, 128], dt))
        psum = ctx.enter_context(nc.psum_tensor([128, FREE_K], dt))

        fd_sem = ctx.enter_context(nc.semaphore("fd"))
        id_sem = ctx.enter_context(nc.semaphore("idw"))
        # One DMA-completion sem per ring slot: completions of different
        # dma_starts on one sem can interleave, so a shared counting sem
        # cannot order slots. Per-slot sems are unambiguous because the
        # consumption sem serializes refills of any single slot.
        k_sems = [ctx.enter_context(nc.semaphore(f"ks{s}")) for s in range(KBUFS)]
        # Per-engine completion counters.
        dve_sem = ctx.enter_context(nc.semaphore("dve_ops"))
        pe_sem = ctx.enter_context(nc.semaphore("pe_taps"))
        odma = ctx.enter_context(nc.semaphore("odma"))
        block = ctx.enter_context(nc.Block())

        feat_v = feat_s[:, :].rearrange("p (hp wp) -> p hp wp", wp=WP)

        def kslot(k):
            s = (k % KBUFS) * FREE_K
            return kbuf[:, s : s + FREE_K]

        def issue_tap_dmas(eng, parity):
            for t in range(KK):
                if t % 2 != parity:
                    continue
                if t >= KBUFS:
                    p = t - KBUFS  # tap that previously used this slot
                    if p in DVE_ACC_TAPS:
                        eng.wait_ge(dve_sem, tap_done[p])
                    else:
                        eng.wait_ge(pe_sem, pe_count[p])
                eng.dma_start(kslot(t), kview[:, :, t, :]).then_inc(
                    k_sems[t % KBUFS], 16
                )

        @block.scalar
        def _(scalar):
            scalar.dma_start(feat_s[:, :], fview).then_inc(fd_sem, 16)
            scalar.dma_start(ident_s[:, :], id_d[:, :]).then_inc(id_sem, 16)
            issue_tap_dmas(scalar, 1)
            scalar.wait_ge(dve_sem, n_dve_ops)
            scalar.dma_start(oview, out_s[:, :]).then_inc(odma, 16)
            scalar.wait_ge(odma, 16)

        @block.sync
        def _(sync):
            issue_tap_dmas(sync, 0)

        @block.vector
        def _(vector):
            vector.wait_ge(fd_sem, 16)
            nops = 0
            for t in range(KK):
                i, j = divmod(t, KSIZE)
                fap = feat_v[:, i : i + HH, j : j + W]
                kt = kslot(t)
                kap = kt.rearrange("p (h w) -> p h w", w=W)
                vector.wait_ge(k_sems[t % KBUFS], 16 * (t // KBUFS + 1))
                if t == first_acc:
                    acc3 = acc_d[:, :].rearrange("p (h w) -> p h w", w=W)
                    vector.tensor_mul(out=acc3, in0=fap, in1=kap).then_inc(dve_sem, 1)
                    nops += 1
                else:
                    vector.tensor_mul(out=kap, in0=fap, in1=kap).then_inc(dve_sem, 1)
                    nops += 1
                    if t in DVE_ACC_TAPS:
                        # same-engine RAW: wait for the multiply first
                        vector.wait_ge(dve_sem, nops)
                        vector.tensor_add(
                            out=acc_d[:, :], in0=acc_d[:, :], in1=kt
                        ).then_inc(dve_sem, 1)
                        nops += 1
                assert nops == tap_done[t], (t, nops, tap_done[t])
            # merge PSUM partial sums, then leaky relu
            vector.wait_ge(pe_sem, n_pe)
            vector.wait_ge(dve_sem, nops)
            vector.tensor_add(
                out=acc_d[:, :], in0=acc_d[:, :], in1=psum[:, :]
            ).then_inc(dve_sem, 1)
            nops += 1
            vector.wait_ge(dve_sem, nops)
            # leaky relu: max(0.2*x, x) in one fused op
            vector.scalar_tensor_tensor(
                out=out_s[:, :],
                in0=acc_d[:, :],
                scalar=0.2,
                in1=acc_d[:, :],
                op0=mybir.AluOpType.mult,
                op1=mybir.AluOpType.max,
            ).then_inc(dve_sem, 1)
            nops += 1
            assert nops == n_dve_ops, (nops, n_dve_ops)

        @block.tensor
        def _(tensor):
            tensor.wait_ge(id_sem, 16)
            for pk, t in enumerate(pe_taps):
                tensor.wait_ge(dve_sem, mult_done[t])
                kt = kslot(t)
                for b in range(n_banks):
                    s = b * PBANK
                    mm = tensor.matmul(
                        out=psum[:, s : s + PBANK],
                        lhsT=ident_s[:, :],
                        rhs=kt[:, s : s + PBANK],
                        start=(pk == 0),
                        stop=(pk == n_pe - 1),
                    )
                mm.then_inc(pe_sem, 1)

    nc.finalize()
    return nc


def _get_nc():
    global _NC
    if _NC is None:
        _NC = _build_nc()
    return _NC


def _shard_inputs(feat_in, kernel):
    in_maps = []
    for core in range(NCORES):
        n, hq = divmod(core, 4)
        h0 = hq * HC
        lo = max(0, h0 - PAD)
        hi = min(H, h0 + HC + PAD)
        top = PAD - (h0 - lo)
        bot = PAD - (hi - (h0 + HC))
        fpad = np.pad(
            feat_in[n, :, lo:hi, :], ((0, 0), (top, bot), (PAD, PAD)), mode="edge"
        )  # (C, HC+4, WP)
        fblocks = np.ascontiguousarray(
            np.stack([fpad[:, hb * HH : hb * HH + HP, :] for hb in range(HB)], axis=1)
        )  # (C, HB, HP, WP)
        kshard = np.ascontiguousarray(kernel[n, :, h0 : h0 + HC, :])  # (C*KK, HC, W)
        in_maps.append(
            {"feat": fblocks, "ker": kshard, "ident": np.eye(128, dtype=np.float32)}
        )
    return in_maps


def kernel(feat_in, kernel, _trace=False, _trace_kwargs=None):
    global LAST_EXEC_NS, LAST_RESULTS
    from concourse.bass_utils import run_bass_kernel_spmd

    feat_in = np.ascontiguousarray(np.asarray(feat_in), dtype=np.float32)
    kernel_np = np.ascontiguousarray(np.asarray(kernel), dtype=np.float32)
    assert feat_in.shape == (N, C, H, W), feat_in.shape
    assert kernel_np.shape == (N, C * KK, H, W), kernel_np.shape

    nc = _get_nc()
    in_maps = _shard_inputs(feat_in, kernel_np)
    kwargs = dict(_trace_kwargs or {})
    res = run_bass_kernel_spmd(
        nc, in_maps, core_ids=list(range(NCORES)), trace=_trace, **kwargs
    )
    LAST_EXEC_NS = res.exec_time_ns
    LAST_RESULTS = res

    out = np.empty((N, C, H, W), dtype=np.float32)
    for core in range(NCORES):
        n, hq = divmod(core, 4)
        h0 = hq * HC
        out[n, :, h0 : h0 + HC, :] = res.results[core]["out"]
    return out
